# revision 4
# baseline (speedup 1.0000x reference)
"""Trainium2 Bass kernel for BrainInspiredEmotionGraph (2-layer RGCN, 17 nodes,
8 relations, d=2048) running SPMD on 8 NeuronCores.

Math: layer(x) = sum_r A_r @ x @ W_r + x @ root + bias, where A_r is the
[17,17] per-relation mean-aggregation matrix built from the edge list.
h1 = relu(layer1(h)); out = layer2(h1), h = node_emb with signal rows patched.

Sharding (fully collective-free):
- Layer 1: output-column sharding. Core c computes h1[:, c*256:(c+1)*256]
  from W1[:, :, chunk] + root1[:, chunk] (host-premixed lhsT: (A_r h)^T per
  relation + h^T for the root, one long PSUM accumulation).
- Layer 2: hidden-dim contraction sharding. Core c computes the partial
  P_c = sum_r (A_r h1[:, chunk]) @ W2_r[chunk, :] + h1[:, chunk] @ root2[chunk, :]
  over the h1 columns it already owns — no inter-core exchange. The host
  sums the 8 [17, 2048] partials and adds bias2.

Performance shape (the problem is pure weight streaming, ~19 MB/core):
- Weights stream as fp16 (half the fp32 bytes; ~5e-4 output rel err) with
  fp32 PSUM accumulation.
- All weights live in SBUF simultaneously ([128, 36864] fp16 per layer),
  so the weight stream is one uninterrupted chunk sequence with no
  buffer-recycle waits, issued as 3 MB DMAs (24 KB per partition per
  trigger, the DMA engines' best regime, ~425 GB/s measured).
- The last layer-2 slab is strip-grouped on the host so the final two
  512 KB chunks each unlock one pair of output strips; outputs ship as
  fp16 with copies+DMA on the vector engine (no cross-engine hop).
"""
import sys

if '/opt/trn_rl_repo' not in sys.path:
    sys.path.insert(0, '/opt/trn_rl_repo')

import numpy as np
from concourse import bacc, tile, mybir, bass_utils

N_NODES = 17
N_REL = 8
D = 2048
N_CORES = 8
CH = D // N_CORES          # 256 columns of h1 owned per core
KT = 128                    # contraction rows per matmul
JT = D // KT                # 16 k-tiles per layer-1 slab
NSTRIP = 4                  # layer-2 output strips of 512 columns
F32 = mybir.dt.float32
F16 = mybir.dt.float16

SLAB = JT * CH              # 4096 fp16 words per slab per partition
WCOLS = 9 * SLAB            # 36864
NX = 9 * JT * N_NODES       # 2448 lhsT columns
# fp32 const-tensor layout (word offsets): A_r^T stack, identity, b1, ones
OFF_AT = 0
OFF_ID = N_REL * N_NODES
OFF_B1 = 160
OFF_ONES = 416
CONSTF_W = 448

# layer-2 slab stream order: root2 first (ready when xt2 is), slab 7 last
# (strip-grouped tail)
ORD2 = (8, 0, 1, 2, 3, 4, 5, 6, 7)

_compiled = None


def _build():
    nc = bacc.Bacc("TRN2", target_bir_lowering=False, debug=False,
                   num_devices=N_CORES)
    # per-partition-contiguous weight planes: w1[p, s*4096 + j*256 + c] is
    # W1full[s, 16p+j, c]; w2[p, i*4096 + kt*2048 + d] is
    # W2full[ORD2[i], kt*128+p, d] (slab 7 internally strip-paired, below)
    w1 = nc.dram_tensor("w1", [KT, WCOLS], F16, kind="ExternalInput").ap()
    w2 = nc.dram_tensor("w2", [KT, WCOLS], F16, kind="ExternalInput").ap()
    xh = nc.dram_tensor("xh", [KT, NX], F16, kind="ExternalInput").ap()
    cf = nc.dram_tensor("cf", [N_NODES, CONSTF_W], F32,
                        kind="ExternalInput").ap()
    out = nc.dram_tensor("out", [KT, NSTRIP * 512], F16,
                         kind="ExternalOutput").ap()

    with tile.TileContext(nc) as tc:
        with tc.tile_pool(name="const", bufs=1) as constp, \
             tc.tile_pool(name="spool", bufs=2) as spool, \
             tc.tile_pool(name="opsum", bufs=1, space="PSUM") as opsum, \
             tc.tile_pool(name="ppsum", bufs=2, space="PSUM") as ppsum:

            xh_sb = constp.tile([KT, NX], F16)
            nc.scalar.dma_start(out=xh_sb, in_=xh)
            cf_sb = constp.tile([N_NODES, CONSTF_W], F32)
            nc.scalar.dma_start(out=cf_sb, in_=cf)
            at_sb = cf_sb[:, OFF_AT:OFF_AT + N_REL * N_NODES]
            id_sb = cf_sb[:, OFF_ID:OFF_ID + N_NODES]
            b1_sb = cf_sb[0:1, OFF_B1:OFF_B1 + CH]
            ones_sb = cf_sb[0:1, OFF_ONES:OFF_ONES + N_NODES]

            # the full weight stream: 3 MB triggers, then the strip-paired
            # 512 KB tail halves of layer-2 slab 7
            w1_sb = constp.tile([KT, WCOLS], F16)
            w2_sb = constp.tile([KT, WCOLS], F16)
            for a, b in ((0, 12288), (12288, 24576), (24576, 36864)):
                nc.sync.dma_start(out=w1_sb[:, a:b], in_=w1[:, a:b])
            for a, b in ((0, 12288), (12288, 24576), (24576, 32768),
                         (32768, 34816), (34816, 36864)):
                nc.sync.dma_start(out=w2_sb[:, a:b], in_=w2[:, a:b])

            def xt(k):
                return xh_sb[:, k * N_NODES:(k + 1) * N_NODES]

            # ---------------- layer 1 ----------------
            # col-tiled: M=17 uses 17 of 128 PE columns, so cycle matmuls
            # through 4 column groups (concurrent on HW); fold strips after.
            out1 = opsum.tile([KT, CH], F32, name="out1")
            started1 = [False] * 4
            mmi1 = [0]
            TOT1 = 1 + 9 * JT

            def l1mm(lhsT, rhs):
                i = mmi1[0]
                g = i % 4
                mmi1[0] += 1
                nc.tensor.matmul(out1[32 * g:32 * g + N_NODES, :],
                                 lhsT=lhsT, rhs=rhs,
                                 start=not started1[g], stop=(i >= TOT1 - 4),
                                 tile_position=(0, 32 * g),
                                 skip_group_check=True)
                started1[g] = True

            for s in range(9):
                for j in range(JT):
                    k = s * JT + j
                    l1mm(xt(k), w1_sb[:, s * SLAB + j * CH:
                                      s * SLAB + (j + 1) * CH])
                if s == 0:
                    l1mm(ones_sb, b1_sb)
            # fold the 4 col-group strips (PSUM inputs may differ in base
            # partition; SB+SB may not)
            t0 = spool.tile([N_NODES, CH], F32, name="t0")
            t1 = spool.tile([N_NODES, CH], F32, name="t1")
            nc.vector.tensor_copy(t0, out1[0:N_NODES, :])
            nc.vector.tensor_add(t1, t0, out1[32:32 + N_NODES, :])
            nc.vector.tensor_add(t0, t1, out1[64:64 + N_NODES, :])
            s01 = spool.tile([N_NODES, CH], F32, name="s01")
            nc.vector.tensor_add(s01, t0, out1[96:96 + N_NODES, :])
            h1 = spool.tile([N_NODES, CH], F32, name="h1")
            nc.scalar.activation(h1, s01, mybir.ActivationFunctionType.Relu)

            # layer-2 lhsT prep: (A_r h1_c)^T for r<8 + h1_c^T for the root,
            # cast to fp16 tiles (indexed by slab id)
            xt2 = spool.tile([KT, 18 * N_NODES], F16, name="xt2")
            for s in range(9):
                rhs = (at_sb[:, s * N_NODES:(s + 1) * N_NODES]
                       if s < N_REL else id_sb)
                for kt in range(2):
                    sl = slice((s * 2 + kt) * N_NODES,
                               (s * 2 + kt + 1) * N_NODES)
                    pp = ppsum.tile([KT, N_NODES], F32, name="pp")
                    nc.tensor.matmul(pp, lhsT=h1[:, kt * KT:(kt + 1) * KT],
                                     rhs=rhs, start=True, stop=True)
                    nc.vector.tensor_copy(xt2[:, sl], pp)

            def lh2(s, kt):
                i = s * 2 + kt
                return xt2[:, i * N_NODES:(i + 1) * N_NODES]

            # ---------------- layer 2 (partial over owned h1 columns) -----
            out2 = []
            started2 = []
            mmi2 = []
            for n in range(NSTRIP):
                out2.append(opsum.tile([KT, 512], F32, name=f"out2_{n}",
                                       tag=f"out2_{n}"))
                started2.append([False] * 4)
                mmi2.append([0])
            TOT2 = 9 * 2

            def l2mm(n, lhsT, rhs):
                i = mmi2[n][0]
                g = (i + n) % 4  # offset by strip: no col-group collision
                mmi2[n][0] += 1
                nc.tensor.matmul(out2[n][32 * g:32 * g + N_NODES, :],
                                 lhsT=lhsT, rhs=rhs,
                                 start=not started2[n][g],
                                 stop=(i >= TOT2 - 4),
                                 tile_position=(0, 32 * g),
                                 skip_group_check=True)
                started2[n][g] = True

            # ship raw [128, 512] col-group partials as fp16; host folds the
            # 4 partition strips. copies + out-DMA stay on the vector engine
            # (no cross-engine hop at the tail).
            osb = spool.tile([KT, NSTRIP * 512], F16, name="osb")

            def strip_out(pair):
                for n in pair:
                    nc.vector.tensor_copy(osb[:, n * 512:(n + 1) * 512],
                                          out2[n])
                a, b = pair[0] * 512, (pair[-1] + 1) * 512
                nc.scalar.dma_start(out=out[:, a:b], in_=osb[:, a:b])

            for i, s in enumerate(ORD2[:8]):
                for kt in range(2):
                    for n in range(NSTRIP):
                        l2mm(n, lh2(s, kt),
                             w2_sb[:, i * SLAB + kt * D + n * 512:
                                   i * SLAB + kt * D + (n + 1) * 512])
            # slab 7 (stream position 8), strip-paired tail: block b holds
            # [kt0 n=2b, kt0 n=2b+1, kt1 n=2b, kt1 n=2b+1] in 512-col units
            for b in range(2):
                base = 8 * SLAB + b * 2048
                for kt in range(2):
                    for n in (2 * b, 2 * b + 1):
                        l2mm(n, lh2(7, kt),
                             w2_sb[:, base + kt * 1024 + (n - 2 * b) * 512:
                                   base + kt * 1024 + (n - 2 * b + 1) * 512])
                strip_out((2 * b, 2 * b + 1))

    nc.compile()
    return nc


def _prep_inputs(inputs):
    """Host-side prep: A matrices, premixed layer-1 lhsT, per-core weights."""
    h = np.array(inputs['node_emb'], dtype=np.float32, copy=True)
    sf = np.asarray(inputs['signal_features'], dtype=np.float32)
    h[:sf.shape[0]] = sf
    src = np.asarray(inputs['edge_index'])[0].astype(np.int64)
    dst = np.asarray(inputs['edge_index'])[1].astype(np.int64)
    et = np.asarray(inputs['edge_type']).astype(np.int64)

    A = np.zeros((N_REL, N_NODES, N_NODES), np.float32)
    cnt = np.zeros((N_REL, N_NODES), np.float32)
    np.add.at(cnt, (et, dst), 1.0)
    np.add.at(A, (et, dst, src), 1.0)
    A /= np.maximum(cnt, 1.0)[:, :, None]

    # layer-1 lhsT: 9 slabs of (A_r h)^T (+ h^T for root), K-permuted so
    # partition p holds rows {16p+j}: [128, 2448] fp16
    Z = np.concatenate([np.einsum('rij,jd->rid', A, h).astype(np.float32),
                        h[None]], axis=0)           # [9,17,2048]
    x1t = (Z.transpose(0, 2, 1)
            .reshape(9, KT, JT, N_NODES)
            .transpose(1, 0, 2, 3)
            .reshape(KT, NX)).astype(np.float16).copy()

    # A_r^T stacked along columns: at[n, r*17+m] = A[r][m, n]
    at = (A.transpose(0, 2, 1).transpose(1, 0, 2)
           .reshape(N_NODES, N_REL * N_NODES)).astype(np.float32)

    W1 = np.asarray(inputs['W1'], dtype=np.float32)
    W2 = np.asarray(inputs['W2'], dtype=np.float32)
    r1 = np.asarray(inputs['root1'], dtype=np.float32)
    r2 = np.asarray(inputs['root2'], dtype=np.float32)
    bias1 = np.asarray(inputs['bias1'], dtype=np.float32)
    W1full = np.concatenate([W1, r1[None]], axis=0)   # [9,2048,2048]
    W2full = np.concatenate([W2, r2[None]], axis=0)   # [9,2048,2048]

    cf = np.zeros((N_NODES, CONSTF_W), np.float32)
    cf[:, OFF_AT:OFF_AT + N_REL * N_NODES] = at
    cf[:, OFF_ID:OFF_ID + N_NODES] = np.eye(N_NODES)
    cf[0, OFF_ONES:OFF_ONES + N_NODES] = 1.0

    in_maps = []
    for c in range(N_CORES):
        cols = slice(c * CH, (c + 1) * CH)
        # w1 plane: [p, s*4096 + j*256 + c] = W1full[s, 16p+j, c]
        w1c = (W1full[:, :, cols]
               .reshape(9, KT, JT, CH)
               .transpose(1, 0, 2, 3)
               .reshape(KT, WCOLS)).astype(np.float16).copy()
        # w2 per-slab planes: [s][p, kt*2048 + d] = W2full[s, kt*128+p, d]
        w2s = (W2full[:, cols, :]
               .reshape(9, 2, KT, D)
               .transpose(0, 2, 1, 3)
               .reshape(9, KT, 2 * D)).astype(np.float16)
        # slab 7 strip-paired: block b = [kt0 n=2b | kt0 n=2b+1 |
        #                                 kt1 n=2b | kt1 n=2b+1]
        s7 = w2s[7].reshape(KT, 2, NSTRIP, 512)       # [p, kt, n, d]
        s7p = np.empty_like(s7).reshape(KT, 2, 2, 2, 512)  # [p, b, kt, nn, d]
        for b in range(2):
            for kt in range(2):
                for nn in range(2):
                    s7p[:, b, kt, nn] = s7[:, kt, 2 * b + nn]
        w2c = np.concatenate(
            [w2s[s] for s in ORD2[:8]] + [s7p.reshape(KT, 2 * D)],
            axis=1).copy()                            # [128, 36864]
        cfc = cf.copy()
        cfc[0, OFF_B1:OFF_B1 + CH] = bias1[cols]
        in_maps.append({
            'w1': w1c,
            'w2': w2c,
            'xh': x1t,
            'cf': cfc,
        })
    return in_maps


def get_compiled():
    global _compiled
    if _compiled is None:
        _compiled = _build()
    return _compiled


def run(inputs, trace=False):
    nc = get_compiled()
    in_maps = _prep_inputs(inputs)
    res = bass_utils.run_bass_kernel_spmd(
        nc, in_maps, core_ids=list(range(N_CORES)), trace=trace)
    acc = np.zeros((N_NODES, D), np.float64)
    for c in range(N_CORES):
        # out[32g+m, n*512+j] = col-group-g partial of P_c[m, n*512+j]
        o = np.asarray(res.results[c]['out'], dtype=np.float64)
        acc += o.reshape(4, 32, D)[:, :N_NODES, :].sum(axis=0)
    acc += np.asarray(inputs['bias2'], dtype=np.float64)[None, :]
    return acc.astype(np.float32), res


def kernel(**inputs):
    outp, _ = run(inputs, trace=False)
    return outp
